# revision 1
# baseline (speedup 1.0000x reference)
"""Trainium2 Bass kernel for nn_CrossAttentionFusion.

Math: softmax over kv_len==1 is identically 1.0, so the attention output is
v broadcast over the N (patch) axis and the whole module reduces to

    out[b, n, :] = cnn[b] @ (Wkv[:, C:] @ Wp) + bp        (independent of n)

W_eff = Wkv[:, C:] @ Wp is a weight-only constant, folded on the host.

Strategy: data-parallel over batch B=64 across 8 NeuronCores (8 batches per
core), W_eff replicated. The 768 output columns are computed in two passes of
384; each pass writes its own contiguous DRAM tensor (outA/outB, concatenated
on the host) so the broadcast DMAs are fully dense. Pass-A weights stream
first (smallest chunk first so the PE starts early); pass-B stage matmuls are
interleaved with pass-A broadcast matmuls. Scratch warm-up matmuls lift the
PE HAM throttle up front. Per (pass, batch) a one-hot matmul replicates
row[b] across 128 SBUF partitions and stride-0-source broadcast DMAs on both
HWDGE rings write the (576, 384) block.
"""

import sys

sys.path.insert(0, "/opt/trn_rl_repo")

import numpy as np

import concourse.bass as bass
import concourse.mybir as mybir
from concourse import bacc
from concourse.bass_utils import run_bass_kernel_spmd
from concourse.tile import TileContext

F32 = mybir.dt.float32

NCORES = 8
B, N, C, CNN = 64, 576, 768, 2048
BS = B // NCORES  # batches per core = 8
KC = CNN // 128  # 16 k-chunks
CW = 384  # columns per pass
# pass-A k-chunk grouping: (n_kchunks, ring); small first chunk on the idle
# scalar ring so the PE starts early while sync streams the bulk
A_GROUPS = ((2, "scalar"), (4, "sync"), (4, "sync"), (4, "sync"), (2, "sync"))


def _build_bass():
    nc = bacc.Bacc(None, target_bir_lowering=False, debug=False, num_devices=NCORES)

    x_cnnT = nc.declare_dram_parameter("cnnT", [128, KC * BS], F32, isOutput=False)
    x_weffA = nc.declare_dram_parameter("weffA", [128, KC * CW], F32, isOutput=False)
    x_weffB = nc.declare_dram_parameter("weffB", [128, KC * CW], F32, isOutput=False)
    x_bpb = nc.declare_dram_parameter("bpb", [BS, C], F32, isOutput=False)
    x_sel = nc.declare_dram_parameter("sel", [BS, BS * 128], F32, isOutput=False)
    yA = nc.declare_dram_parameter("outA", [BS, N, CW], F32, isOutput=True)
    yB = nc.declare_dram_parameter("outB", [BS, N, CW], F32, isOutput=True)

    with TileContext(nc) as tc:
        with (
            tc.tile_pool(name="singles", bufs=1) as singles,
            tc.tile_pool(name="psum_r", bufs=1, space="PSUM") as psum_r,
            tc.tile_pool(name="psum_bc", bufs=5, space="PSUM") as psum_bc,
            tc.tile_pool(name="bc_sb", bufs=8) as bc_sb,
        ):
            # PE warm-up: junk matmuls on scratch data lift the HAM throttle
            # (~3.4 us busy window) before the real matmuls arrive.
            wu_sb = singles.tile([128, 512], F32, tag="wu_sb")
            nc.gpsimd.memset(wu_sb[:], 0.0)
            with tc.tile_pool(name="psum_w", bufs=1, space="PSUM") as psum_w:
                ps_w = psum_w.tile([BS, 512], F32, tag="ps_w")
                nc.tensor.matmul(
                    ps_w[:], wu_sb[:, 0:BS], wu_sb[:, :], start=True, stop=True
                )

            # cnnT and the first weight chunk ride the otherwise-idle scalar
            # ring so the PE can start while the sync ring streams the rest.
            cnnT_t = singles.tile([128, KC * BS], F32, tag="cnnT")
            nc.scalar.dma_start(out=cnnT_t[:], in_=x_cnnT[:, :])
            weffA_t = []
            kc0 = 0
            for gi, (gk, eng) in enumerate(A_GROUPS):
                wt = singles.tile(
                    [128, gk * CW], F32, tag=f"weffA{gi}", name=f"weffA{gi}"
                )
                eng = nc.scalar if eng == "scalar" else nc.sync
                eng.dma_start(out=wt[:], in_=x_weffA[:, kc0 * CW : (kc0 + gk) * CW])
                weffA_t.append((kc0, gk, wt))
                kc0 += gk
            weffB_t = []
            for g in range(4):
                wt = singles.tile([128, 4 * CW], F32, tag=f"weffB{g}", name=f"weffB{g}")
                nc.sync.dma_start(
                    out=wt[:], in_=x_weffB[:, g * 4 * CW : (g + 1) * 4 * CW]
                )
                weffB_t.append((4 * g, 4, wt))
            sel_t = singles.tile([BS, BS * 128], F32, tag="sel")
            nc.scalar.dma_start(out=sel_t[:], in_=x_sel[:, :])
            bpb_t = singles.tile([BS, C], F32, tag="bpb")
            nc.scalar.dma_start(out=bpb_t[:], in_=x_bpb[:, :])

            row_t = singles.tile([BS, C], F32, tag="row")
            ps_rowA = psum_r.tile([BS, CW], F32, tag="ps_rowA", name="ps_rowA")
            ps_rowB = psum_r.tile([BS, CW], F32, tag="ps_rowB", name="ps_rowB")

            def stage_group(ps_row, group):
                kc0, gk, wt = group
                for i in range(gk):
                    kc = kc0 + i
                    nc.tensor.matmul(
                        ps_row[:],
                        cnnT_t[:, kc * BS : (kc + 1) * BS],
                        wt[:, i * CW : (i + 1) * CW],
                        start=(kc == 0),
                        stop=(kc == KC - 1),
                    )

            def bcast(b, half):
                c0 = half * CW
                yy = yA if half == 0 else yB
                ps_bc = psum_bc.tile([128, CW], F32, name="ps_bc", tag="ps_bc")
                nc.tensor.matmul(
                    ps_bc[:],
                    sel_t[:, b * 128 : (b + 1) * 128],
                    row_t[:, c0 : c0 + CW],
                    start=True,
                    stop=True,
                )
                bc_t = bc_sb.tile([128, CW], F32, name="bc_t", tag="bc_t")
                nc.vector.tensor_copy(bc_t[:], ps_bc[:])

                # rows 0..511: n = 4*p + j, 128 partitions, stride-0 j.
                src_a = bc_t[:, :].unsqueeze(1).broadcast_to((128, 4, CW))
                dst_a = yy[b, 0:512, :].rearrange("(p j) c -> p j c", j=4)
                # rows 512..575 from 64 partitions (alternate halves).
                h0 = 0 if b % 2 == 0 else 64
                src_b = bc_t[h0 : h0 + 64, :]
                dst_b = yy[b, 512:N, :]
                eng_a = nc.sync if b % 2 == 0 else nc.scalar
                eng_b = nc.scalar if b % 2 == 0 else nc.sync
                eng_a.dma_start(out=dst_a, in_=src_a)
                eng_b.dma_start(out=dst_b, in_=src_b)

            # Pass A stage, then its bias add.
            for group in weffA_t:
                stage_group(ps_rowA, group)
            nc.vector.tensor_add(row_t[:, 0:CW], ps_rowA[:], bpb_t[:, 0:CW])

            # Interleave pass-A broadcasts with pass-B stage matmuls so the
            # out-DMA stream never starves while pass B computes.
            bcast(0, 0)
            bcast(1, 0)
            for g in range(4):
                stage_group(ps_rowB, weffB_t[g])
                bcast(2 + g, 0)
            bcast(6, 0)
            bcast(7, 0)
            nc.vector.tensor_add(row_t[:, CW:C], ps_rowB[:], bpb_t[:, CW:C])
            for b in range(BS):
                bcast(b, 1)

    nc.compile()
    return nc


_NC = None


def _get_nc():
    global _NC
    if _NC is None:
        _NC = _build_bass()
    return _NC


def _prepare_in_maps(image_patches, cnn_feature_vector, Wq, Wkv, Wp, bp):
    Weff = np.ascontiguousarray(Wkv[:, C:]) @ Wp  # (2048, 768) fp32
    weffA_arr = np.ascontiguousarray(
        Weff[:, 0:CW].reshape(KC, 128, CW).transpose(1, 0, 2).reshape(128, KC * CW)
    )
    weffB_arr = np.ascontiguousarray(
        Weff[:, CW:C].reshape(KC, 128, CW).transpose(1, 0, 2).reshape(128, KC * CW)
    )
    bpb = np.ascontiguousarray(np.broadcast_to(bp.astype(np.float32), (BS, C)))
    sel = np.zeros((BS, BS * 128), dtype=np.float32)
    for b in range(BS):
        sel[b, b * 128 : (b + 1) * 128] = 1.0

    in_maps = []
    for core in range(NCORES):
        shard = cnn_feature_vector[core * BS : (core + 1) * BS]  # (8, 2048)
        cnnT = np.ascontiguousarray(
            shard.T.reshape(KC, 128, BS).transpose(1, 0, 2).reshape(128, KC * BS)
        )
        in_maps.append(
            {
                "cnnT": cnnT,
                "weffA": weffA_arr,
                "weffB": weffB_arr,
                "bpb": bpb,
                "sel": sel,
            }
        )
    return in_maps


def _assemble(res):
    out = np.empty((B, N, C), dtype=np.float32)
    for i in range(NCORES):
        sl = slice(i * BS, (i + 1) * BS)
        out[sl, :, 0:CW] = res.results[i]["outA"]
        out[sl, :, CW:C] = res.results[i]["outB"]
    return out


def kernel(**inputs) -> np.ndarray:
    inputs = {k: np.asarray(v) for k, v in inputs.items()}
    nc = _get_nc()
    in_maps = _prepare_in_maps(**inputs)
    res = run_bass_kernel_spmd(nc, in_maps, core_ids=list(range(NCORES)))
    return _assemble(res)


def kernel_traced(**inputs):
    """kernel() + HW profile; returns (output, BassKernelResults)."""
    inputs = {k: np.asarray(v) for k, v in inputs.items()}
    nc = _get_nc()
    in_maps = _prepare_in_maps(**inputs)
    res = run_bass_kernel_spmd(
        nc, in_maps, core_ids=list(range(NCORES)), trace=True
    )
    return _assemble(res), res



# revision 2
# speedup vs baseline: 1.1936x; 1.1936x over previous
"""Trainium2 Bass kernel for nn_CrossAttentionFusion.

Math: softmax over kv_len==1 is identically 1.0, so the attention output is
v broadcast over the N (patch) axis and the whole module reduces to

    out[b, n, :] = cnn[b] @ (Wkv[:, C:] @ Wp) + bp        (independent of n)

W_eff = Wkv[:, C:] @ Wp is a weight-only constant, folded on the host.

Sharding: 8 cores = 4 batch-groups x 2 column-groups. Each core computes
y = cnn_shard @ W_eff_slice + bp_slice for its 16 batches x 384 columns and
writes the [16, 576, 384] output block (14.16 MB; the kernel is bound by
this HBM write stream).

Bandwidth tricks vs the fp32 data-parallel baseline:
  * weights and activations stream in bf16 (error ~1e-3 << 2e-2 gate),
    cutting read traffic from 6.4 MB to ~2.1 MB per core;
  * the cnn shard is host-replicated 8x along the M axis of the lhsT so the
    K-chunk accumulation produces y directly REPLICATED across all 128 PSUM
    partitions (partition p holds y[p//8]) - no one-hot broadcast matmul;
  * the bias rides as a K=1 fp32 accumulation chunk issued before the
    weights even arrive;
  * output writes are 9 DMAs of 1.57 MB (stride-0 source: partition p
    writes its 1536B row 8 times = 8 output rows), split across both HWDGE
    rings so the write stream starts ~1 us after the last weight chunk.
"""

import sys

sys.path.insert(0, "/opt/trn_rl_repo")

import ml_dtypes
import numpy as np

import concourse.bass as bass
import concourse.mybir as mybir
from concourse import bacc
from concourse.bass_utils import run_bass_kernel_spmd
from concourse.tile import TileContext

F32 = mybir.dt.float32
BF16 = mybir.dt.bfloat16
NPBF16 = np.dtype(ml_dtypes.bfloat16)

NCORES = 8
B, N, C, CNN = 64, 576, 768, 2048
BGROUPS, CGROUPS = 4, 2          # batch groups x column groups
BS = B // BGROUPS                # 16 batches per core
CW = C // CGROUPS                # 384 columns per core
KC = CNN // 128                  # 16 k-chunks
REP = 128 // BS                  # 8 partitions per batch
ROWS_PP = N // REP               # 72 output rows per partition
RPT = 8                          # rows per partition per write DMA
NWR = ROWS_PP // RPT             # 9 write DMAs
# weight k-chunk grouping across the two rings (sync gets the head, scalar
# tail) so the last chunk lands as early as possible
W_GROUPS_SYNC = (6, 6)
W_GROUPS_SCAL = (4,)


def _build_bass():
    nc = bacc.Bacc(None, target_bir_lowering=False, debug=False, num_devices=NCORES)

    x_cnn = nc.declare_dram_parameter("cnnrep", [128, KC * 128], BF16, isOutput=False)
    x_weff = nc.declare_dram_parameter("weff", [128, KC * CW], BF16, isOutput=False)
    x_bias = nc.declare_dram_parameter("biaspack", [1, 128 + CW], F32, isOutput=False)
    y = nc.declare_dram_parameter("out", [BS, N, CW], F32, isOutput=True)

    with TileContext(nc) as tc:
        with (
            tc.tile_pool(name="singles", bufs=1) as singles,
            tc.tile_pool(name="psum_y", bufs=1, space="PSUM") as psum_y,
        ):
            # PE warm-up: junk matmul on scratch data lifts the HAM throttle
            # before the real (latency-critical) matmuls arrive.
            wu_sb = singles.tile([128, 512], F32, tag="wu_sb")
            nc.gpsimd.memset(wu_sb[:], 0.0)
            with tc.tile_pool(name="psum_w", bufs=1, space="PSUM") as psum_w:
                ps_w = psum_w.tile([8, 512], F32, tag="ps_w")
                nc.tensor.matmul(
                    ps_w[:], wu_sb[:, 0:8], wu_sb[:, :], start=True, stop=True
                )

            bias_t = singles.tile([1, 128 + CW], F32, tag="bias")
            nc.scalar.dma_start(out=bias_t[:], in_=x_bias[:, :])
            cnn_t = singles.tile([128, KC * 128], BF16, tag="cnn")
            half = KC * 128 // 2
            nc.scalar.dma_start(out=cnn_t[:, 0:half], in_=x_cnn[:, 0:half])
            nc.scalar.dma_start(out=cnn_t[:, half:], in_=x_cnn[:, half:])

            weff_t = singles.tile([128, KC * CW], BF16, tag="weff")
            kc0 = 0
            w_groups = []
            for gk in W_GROUPS_SYNC:
                w_groups.append((kc0, gk, nc.sync))
                kc0 += gk
            for gk in W_GROUPS_SCAL:
                w_groups.append((kc0, gk, nc.scalar))
                kc0 += gk
            for kcs, gk, eng in w_groups:
                eng.dma_start(
                    out=weff_t[:, kcs * CW : (kcs + gk) * CW],
                    in_=x_weff[:, kcs * CW : (kcs + gk) * CW],
                )

            # y replicated: ps_y[p, c] = bp[c] + sum_k cnn[p//8, k]*Weff[k, c]
            ps_y = psum_y.tile([128, CW], F32, tag="ps_y")
            nc.tensor.matmul(
                ps_y[:],
                bias_t[:, 0:128],
                bias_t[:, 128 : 128 + CW],
                start=True,
                stop=False,
            )
            for kc in range(KC):
                nc.tensor.matmul(
                    ps_y[:],
                    cnn_t[:, kc * 128 : (kc + 1) * 128],
                    weff_t[:, kc * CW : (kc + 1) * CW],
                    start=False,
                    stop=(kc == KC - 1),
                )

            bc_t = singles.tile([128, CW], F32, tag="bc")
            nc.vector.tensor_copy(bc_t[:], ps_y[:])

            # out rows n = q*72 + s for partition p = b*8 + q; each DMA
            # writes RPT consecutive rows per partition (stride-0 source).
            y_v = y.rearrange("b (q s) c -> (b q) s c", q=REP)
            src = bc_t[:, :].unsqueeze(1).broadcast_to((128, RPT, CW))
            for i in range(NWR):
                eng = nc.sync if i % 2 == 0 else nc.scalar
                eng.dma_start(out=y_v[:, i * RPT : (i + 1) * RPT, :], in_=src)

    nc.compile()
    return nc


_NC = None


def _get_nc():
    global _NC
    if _NC is None:
        _NC = _build_bass()
    return _NC


def _prepare_in_maps(image_patches, cnn_feature_vector, Wq, Wkv, Wp, bp):
    Weff = np.ascontiguousarray(Wkv[:, C:]) @ Wp  # (2048, 768) fp32
    bp = bp.astype(np.float32)

    weff_arrs = []
    bias_arrs = []
    for cg in range(CGROUPS):
        sl = slice(cg * CW, (cg + 1) * CW)
        weff_arrs.append(
            np.ascontiguousarray(
                Weff[:, sl]
                .reshape(KC, 128, CW)
                .transpose(1, 0, 2)
                .reshape(128, KC * CW)
                .astype(NPBF16)
            )
        )
        pack = np.empty((1, 128 + CW), dtype=np.float32)
        pack[0, :128] = 1.0
        pack[0, 128:] = bp[sl]
        bias_arrs.append(pack)

    cnn_arrs = []
    for bg in range(BGROUPS):
        shard = cnn_feature_vector[bg * BS : (bg + 1) * BS]  # (16, 2048)
        rep = np.repeat(shard, REP, axis=0)  # (128, 2048), row p = batch p//8
        cnn_arrs.append(
            np.ascontiguousarray(
                rep.reshape(128, KC, 128)
                .transpose(2, 1, 0)
                .reshape(128, KC * 128)
                .astype(NPBF16)
            )
        )

    in_maps = []
    for core in range(NCORES):
        bg, cg = core // CGROUPS, core % CGROUPS
        in_maps.append(
            {
                "cnnrep": cnn_arrs[bg],
                "weff": weff_arrs[cg],
                "biaspack": bias_arrs[cg],
            }
        )
    return in_maps


def _assemble(res):
    out = np.empty((B, N, C), dtype=np.float32)
    for core in range(NCORES):
        bg, cg = core // CGROUPS, core % CGROUPS
        out[bg * BS : (bg + 1) * BS, :, cg * CW : (cg + 1) * CW] = res.results[
            core
        ]["out"]
    return out


def kernel(**inputs) -> np.ndarray:
    inputs = {k: np.asarray(v) for k, v in inputs.items()}
    nc = _get_nc()
    in_maps = _prepare_in_maps(**inputs)
    res = run_bass_kernel_spmd(nc, in_maps, core_ids=list(range(NCORES)))
    return _assemble(res)


def kernel_traced(**inputs):
    """kernel() + HW profile; returns (output, BassKernelResults)."""
    inputs = {k: np.asarray(v) for k, v in inputs.items()}
    nc = _get_nc()
    in_maps = _prepare_in_maps(**inputs)
    res = run_bass_kernel_spmd(
        nc, in_maps, core_ids=list(range(NCORES)), trace=True
    )
    return _assemble(res), res


# revision 3
# speedup vs baseline: 1.3435x; 1.1256x over previous
"""Trainium2 Bass kernel for nn_CrossAttentionFusion.

Math: softmax over kv_len==1 is identically 1.0, so the attention output is
v broadcast over the N (patch) axis and the whole module reduces to

    out[b, n, :] = cnn[b] @ (Wkv[:, C:] @ Wp) + bp        (independent of n)

W_eff = Wkv[:, C:] @ Wp is a weight-only constant, folded on the host.

Sharding: 8 cores = 4 batch-groups x 2 column-groups. Each core computes
y = cnn_shard @ W_eff_slice + bp_slice for its 16 batches x 384 columns and
writes the [16, 576, 384] output block (14.16 MB; the kernel is bound by
this HBM write stream).

Bandwidth tricks vs the fp32 data-parallel baseline:
  * weights and activations stream in bf16 (error ~2e-3 << 2e-2 gate),
    cutting read traffic from 6.4 MB to ~2.1 MB per core;
  * the cnn shard is host-replicated 8x along the M axis of the lhsT so the
    K-chunk accumulation produces y directly REPLICATED across all 128 PSUM
    partitions (partition p holds y[p//8]) - no one-hot broadcast matmul;
  * the bias rides as a cheap bf16 K=1 accumulation chunk;
  * weights stream in 8 groups of 2 k-chunks so the PE trails the DMA by
    ~one group instead of waiting for the whole tensor (DMA completion
    sems fire near the end of the read phase when transfers are large);
  * the replicated row is materialized 4x in SBUF (bc4) so the output
    DMAs carry 6144B descriptors; the first two writes source from the
    partial bc4 prefix (1536/3072B descs) to start the write stream while
    the remaining copies land.
"""

import sys

sys.path.insert(0, "/opt/trn_rl_repo")

import ml_dtypes
import numpy as np

import concourse.bass as bass
import concourse.mybir as mybir
from concourse import bacc
from concourse.bass_utils import run_bass_kernel_spmd
from concourse.tile import TileContext

F32 = mybir.dt.float32
BF16 = mybir.dt.bfloat16
NPBF16 = np.dtype(ml_dtypes.bfloat16)

NCORES = 8
B, N, C, CNN = 64, 576, 768, 2048
BGROUPS, CGROUPS = 4, 2          # batch groups x column groups
BS = B // BGROUPS                # 16 batches per core
CW = C // CGROUPS                # 384 columns per core
KC = CNN // 128                  # 16 k-chunks
REP = 128 // BS                  # 8 partitions per batch
ROWS_PP = N // REP               # 72 output rows per partition
RPT = 8                          # rows per partition per write DMA
NWR = ROWS_PP // RPT             # 9 write DMAs
WG = 2                           # weight k-chunks per DMA group
NCOPIES = 4                      # replicated row copies in SBUF (desc size)


def _build_bass():
    nc = bacc.Bacc(None, target_bir_lowering=False, debug=False, num_devices=NCORES)

    x_cnn = nc.declare_dram_parameter("cnnrep", [128, KC * 128], BF16, isOutput=False)
    x_weff = nc.declare_dram_parameter("weff", [128, KC * CW], BF16, isOutput=False)
    x_bias = nc.declare_dram_parameter("biaspack", [1, 128 + CW], BF16, isOutput=False)
    y = nc.declare_dram_parameter("out", [BS, N, CW], F32, isOutput=True)

    with TileContext(nc) as tc:
        with (
            tc.tile_pool(name="singles", bufs=1) as singles,
            tc.tile_pool(name="psum_y", bufs=1, space="PSUM") as psum_y,
        ):
            # PE warm-up: junk matmul on scratch data ramps the HAM di/dt
            # throttle before the latency-critical matmuls arrive.
            wu_sb = singles.tile([128, 512], F32, tag="wu_sb")
            nc.gpsimd.memset(wu_sb[:], 0.0)
            with tc.tile_pool(name="psum_w", bufs=1, space="PSUM") as psum_w:
                ps_w = psum_w.tile([8, 512], F32, tag="ps_w")
                nc.tensor.matmul(
                    ps_w[:], wu_sb[:, 0:8], wu_sb[:, :], start=True, stop=True
                )

            bias_t = singles.tile([1, 128 + CW], BF16, tag="bias")
            nc.scalar.dma_start(out=bias_t[:], in_=x_bias[:, :])
            cnn_t = singles.tile([128, KC * 128], BF16, tag="cnn")
            half = KC * 128 // 2
            nc.scalar.dma_start(out=cnn_t[:, 0:half], in_=x_cnn[:, 0:half])
            nc.scalar.dma_start(out=cnn_t[:, half:], in_=x_cnn[:, half:])

            # weights in small groups so completion sems fire incrementally
            weff_t = singles.tile([128, KC * CW], BF16, tag="weff")
            for g in range(KC // WG):
                nc.sync.dma_start(
                    out=weff_t[:, g * WG * CW : (g + 1) * WG * CW],
                    in_=x_weff[:, g * WG * CW : (g + 1) * WG * CW],
                )

            # y replicated: ps_y[p, c] = bp[c] + sum_k cnn[p//8, k]*Weff[k, c]
            ps_y = psum_y.tile([128, CW], F32, tag="ps_y")
            nc.tensor.matmul(
                ps_y[:],
                bias_t[:, 0:128],
                bias_t[:, 128 : 128 + CW],
                start=True,
                stop=False,
            )
            for kc in range(KC):
                nc.tensor.matmul(
                    ps_y[:],
                    cnn_t[:, kc * 128 : (kc + 1) * 128],
                    weff_t[:, kc * CW : (kc + 1) * CW],
                    start=False,
                    stop=(kc == KC - 1),
                )

            # materialize NCOPIES of the row per partition for fat write descs
            bc4 = singles.tile([128, NCOPIES * CW], F32, tag="bc4")
            for j in range(NCOPIES):
                nc.vector.tensor_copy(bc4[:, j * CW : (j + 1) * CW], ps_y[:])

            # out rows n = q*72 + s for partition p = b*8 + q; each DMA
            # writes RPT consecutive rows per partition. Sources grow with
            # the bc4 prefix so early writes launch before all copies land.
            y_v = y.rearrange("b (q s) c -> (b q) s c", q=REP)
            srcs = {
                0: bc4[:, 0:CW].unsqueeze(1).broadcast_to((128, RPT, CW)),
                1: bc4[:, 0 : 2 * CW]
                .unsqueeze(1)
                .broadcast_to((128, RPT // 2, 2 * CW)),
            }
            src_full = (
                bc4[:, :]
                .unsqueeze(1)
                .broadcast_to((128, RPT // NCOPIES, NCOPIES * CW))
            )
            for i in range(NWR):
                eng = nc.sync if i % 2 == 0 else nc.scalar
                eng.dma_start(
                    out=y_v[:, i * RPT : (i + 1) * RPT, :],
                    in_=srcs.get(i, src_full),
                )

    nc.compile()
    return nc


_NC = None


def _get_nc():
    global _NC
    if _NC is None:
        _NC = _build_bass()
    return _NC


def _prepare_in_maps(image_patches, cnn_feature_vector, Wq, Wkv, Wp, bp):
    Weff = np.ascontiguousarray(Wkv[:, C:]) @ Wp  # (2048, 768) fp32
    bp = bp.astype(np.float32)

    weff_arrs = []
    bias_arrs = []
    for cg in range(CGROUPS):
        sl = slice(cg * CW, (cg + 1) * CW)
        weff_arrs.append(
            np.ascontiguousarray(
                Weff[:, sl]
                .reshape(KC, 128, CW)
                .transpose(1, 0, 2)
                .reshape(128, KC * CW)
                .astype(NPBF16)
            )
        )
        pack = np.empty((1, 128 + CW), dtype=np.float32)
        pack[0, :128] = 1.0
        pack[0, 128:] = bp[sl]
        bias_arrs.append(pack.astype(NPBF16))

    cnn_arrs = []
    for bg in range(BGROUPS):
        shard = cnn_feature_vector[bg * BS : (bg + 1) * BS]  # (16, 2048)
        rep = np.repeat(shard, REP, axis=0)  # (128, 2048), row p = batch p//8
        cnn_arrs.append(
            np.ascontiguousarray(
                rep.reshape(128, KC, 128)
                .transpose(2, 1, 0)
                .reshape(128, KC * 128)
                .astype(NPBF16)
            )
        )

    in_maps = []
    for core in range(NCORES):
        bg, cg = core // CGROUPS, core % CGROUPS
        in_maps.append(
            {
                "cnnrep": cnn_arrs[bg],
                "weff": weff_arrs[cg],
                "biaspack": bias_arrs[cg],
            }
        )
    return in_maps


def _assemble(res):
    out = np.empty((B, N, C), dtype=np.float32)
    for core in range(NCORES):
        bg, cg = core // CGROUPS, core % CGROUPS
        out[bg * BS : (bg + 1) * BS, :, cg * CW : (cg + 1) * CW] = res.results[
            core
        ]["out"]
    return out


def kernel(**inputs) -> np.ndarray:
    inputs = {k: np.asarray(v) for k, v in inputs.items()}
    nc = _get_nc()
    in_maps = _prepare_in_maps(**inputs)
    res = run_bass_kernel_spmd(nc, in_maps, core_ids=list(range(NCORES)))
    return _assemble(res)


def kernel_traced(**inputs):
    """kernel() + HW profile; returns (output, BassKernelResults)."""
    inputs = {k: np.asarray(v) for k, v in inputs.items()}
    nc = _get_nc()
    in_maps = _prepare_in_maps(**inputs)
    res = run_bass_kernel_spmd(
        nc, in_maps, core_ids=list(range(NCORES)), trace=True
    )
    return _assemble(res), res
